# revision 17
# baseline (speedup 1.0000x reference)
"""DigitCaps dynamic-routing kernel for 8 Trainium2 NeuronCores.

Strategy (batch-sharded, fully local per core, no collectives):
  B=256 -> 8 cores x 32 batch rows. Each core computes u_hat for its batch
  shard with the PE (4 diagonal 32x32 tiles per round: i%4 -> row/col group,
  contraction d=8), keeps u_hat resident in SBUF as bf16 in layout
      u_hat[p = 32*(i%4) + b, free = (i//4)*160 + k*10 + o]
  and runs the 3 routing iterations on-chip:
    iter1: c uniform -> s1 = 0.1*sum_i u_hat + bias via a dense PE matmul
           over the joint (i,d)=9216 contraction (host-packed dense layouts).
    agreement passes: DVE bf16 2x multiply (V broadcast) + halving-tree sum
           over k; logits/softmax per (b,i) are partition-local.
    s passes: DVE multiply by c (broadcast over k) + halving tree over i//4,
           then a ones-block-diagonal PE matmul folds the 4 i%4 partition
           groups; squash is computed on [32,160] tiles.
  Host pre-packs x/W into the exact SBUF images (transpose+pad in numpy) so
  every DMA is a plain contiguous copy.
"""

import numpy as np

B, NI, DI, NO, K = 256, 1152, 8, 10, 16
NC = 8
BL = B // NC            # 32 batch rows per core
NJ = NI // 4            # 288 j-rounds (i = 4*j + r)
OK = NO * K             # 160, stored in (k, o) order: idx = k*10 + o
NQ = NI // 16           # 72 dense chunks (i = 16*q + t)
EPS = 1e-9

# creation PSUM chunking: 9 j-rounds per psum tile, 3 j per 512-col bank slot
JC = 9
# DVE block size (j per block) for routing passes
JB = 32

_CACHE = {}


def _pack_inputs(x, W, bias):
    """Host-side packing into per-core DMA images (all plain contiguous)."""
    import ml_dtypes
    bf16 = ml_dtypes.bfloat16

    # xt (padded transpose, creation lhsT): [32, NJ*32] rows=(r,d) dense 32
    #   xt[8*r + d, j*32 + b] = x[b0+b, 4*j+r, d]
    # stored dense [32 rows, NJ*BL]; DMA'd per r-group into partitions 32r..32r+8
    xr = x.reshape(B, NJ, 4, DI)                       # [b, j, r, d]
    xt_all = np.ascontiguousarray(
        xr.transpose(2, 3, 1, 0)).astype(bf16)         # [r, d, j, b_all]

    # dense xtD (s1 lhsT): [128, NQ*32]: xtD[16*?] rows = (t, d) = 128
    xd = x.reshape(B, NQ, 16, DI)                      # [b, q, t, d]
    xtD_all = np.ascontiguousarray(
        xd.transpose(2, 3, 1, 0)).astype(bf16)         # [t, d, q, b_all]

    # wp (creation rhs stream): dense rows [32, NJ*OK]
    #   wp[8*r + d, j*160 + k*10 + o] = W[4*j+r, o, k, d]
    wr = W.reshape(NJ, 4, NO, K, DI)                   # [j, r, o, k, d]
    wp = np.ascontiguousarray(
        wr.transpose(1, 4, 0, 3, 2).reshape(32, NJ * OK)).astype(bf16)

    # wd (s1 rhs stream, dense): [128, NQ*OK]
    #   wd[8*t + d, q*160 + k*10 + o] = W[16*q+t, o, k, d]
    wq = W.reshape(NQ, 16, NO, K, DI)                  # [q, t, o, k, d]
    wd = np.ascontiguousarray(
        wq.transpose(1, 4, 0, 3, 2).reshape(128, NQ * OK)).astype(bf16)

    # ones block-diag for folding 4 partition groups: [128, 32]
    ones_bd = np.zeros((128, BL), dtype=np.float32)
    for c in range(4):
        ones_bd[np.arange(BL) + 32 * c, np.arange(BL)] = 1.0
    ones_bd = ones_bd.astype(bf16)

    # onesT for partition replication via PE: [32, 128]
    onesT = np.zeros((BL, 128), dtype=np.float32)
    for c in range(4):
        onesT[np.arange(BL), np.arange(BL) + 32 * c] = 1.0
    onesT = onesT.astype(bf16)

    # bias replicated [32, 160] f32 in (k, o) order
    biasr = np.ascontiguousarray(
        np.broadcast_to(bias.T.reshape(1, OK), (BL, OK))).astype(np.float32)

    per_core = []
    for cid in range(NC):
        b0 = cid * BL
        xt = np.ascontiguousarray(
            xt_all[:, :, :, b0:b0 + BL].reshape(32, NJ * BL))
        xtD = np.ascontiguousarray(
            xtD_all[:, :, :, b0:b0 + BL].reshape(128, NQ * BL))
        per_core.append({
            "xt": xt, "xtd": xtD, "wp": wp, "wd": wd,
            "ones_bd": ones_bd, "onesT": onesT, "biasr": biasr,
        })
    return per_core


def _build_bass(debug=False):
    import concourse.bass as bass
    import concourse.bacc as bacc
    import concourse.mybir as mybir
    from concourse.tile import TileContext

    dt = mybir.dt
    ALU = mybir.AluOpType
    ACTF = mybir.ActivationFunctionType
    AX = mybir.AxisListType

    nc = bacc.Bacc()

    xt_d = nc.dram_tensor("xt", [32, NJ * BL], dt.bfloat16, kind="ExternalInput")
    xtD_d = nc.dram_tensor("xtd", [128, NQ * BL], dt.bfloat16, kind="ExternalInput")
    wp_d = nc.dram_tensor("wp", [32, NJ * OK], dt.bfloat16, kind="ExternalInput")
    wd_d = nc.dram_tensor("wd", [128, NQ * OK], dt.bfloat16, kind="ExternalInput")
    ones_d = nc.dram_tensor("ones_bd", [128, BL], dt.bfloat16, kind="ExternalInput")
    onesT_d = nc.dram_tensor("onesT", [BL, 128], dt.bfloat16, kind="ExternalInput")
    biasr_d = nc.dram_tensor("biasr", [BL, OK], dt.float32, kind="ExternalInput")
    out_d = nc.dram_tensor("out_v", [BL, OK], dt.float32, kind="ExternalOutput")
    if debug:
        dbg_u = nc.dram_tensor("dbg_u", [128, NJ * OK], dt.bfloat16, kind="ExternalOutput")
        dbg_L = nc.dram_tensor("dbg_L", [128, NJ * NO], dt.float32, kind="ExternalOutput")
        dbg_v1 = nc.dram_tensor("dbg_v1", [128, OK], dt.bfloat16, kind="ExternalOutput")


    with TileContext(nc) as tc:
        with (
            tc.tile_pool(name="const", bufs=1) as const,
            tc.tile_pool(name="big", bufs=1) as big,
            tc.tile_pool(name="xw", bufs=4) as xw,
            tc.tile_pool(name="tmp", bufs=2) as tmpp,
            tc.tile_pool(name="small", bufs=2) as small,
            tc.tile_pool(name="ps", bufs=2, space="PSUM") as psp,
            tc.tile_pool(name="pss", bufs=1, space="PSUM") as pss,
        ):
            # ---- resident tiles ----
            u_hat = big.tile([128, NJ * OK], dt.bfloat16)     # 92KB/part
            L = big.tile([128, NJ * NO], dt.float32)          # logits [p,(j,o)]
            eL = big.tile([128, NJ * NO], dt.float32)         # exp(L)
            c16 = big.tile([128, NJ * NO], dt.bfloat16)       # softmax out
            xtD_sb = const.tile([128, NQ * BL], dt.bfloat16)
            ones_sb = const.tile([128, BL], dt.bfloat16)
            onesT_sb = const.tile([BL, 128], dt.bfloat16)
            biasr_sb = const.tile([BL, OK], dt.float32)
            vrep = const.tile([128, OK], dt.bfloat16)         # V replicated x4
            sacc = const.tile([128, OK], dt.float32)          # s partial (c~,b)
            sacc16 = const.tile([128, OK], dt.bfloat16)
            zr = const.tile([128, NJ], dt.float32)

            nc.sync.dma_start(xtD_sb[:, :], xtD_d[:, :])
            touch = const.tile([1, 4], dt.float32)
            nc.sync.dma_start(ones_sb[:, :], ones_d[:, :])
            nc.sync.dma_start(onesT_sb[:, :], onesT_d[:, :])
            nc.sync.dma_start(biasr_sb[:, :], biasr_d[:, :])

            # =========== s1: dense (i,d) contraction ===========
            wd_sb = const.tile([128, NQ * OK], dt.bfloat16)
            nc.sync.dma_start(wd_sb[:, :], wd_d[:, :])
            s1_ps = pss.tile([BL, OK], dt.float32, tag="sps")
            for q in range(NQ):
                nc.tensor.matmul(
                    s1_ps[:, :],
                    xtD_sb[:, q * BL:(q + 1) * BL],
                    wd_sb[:, q * OK:(q + 1) * OK],
                    start=(q == 0), stop=(q == NQ - 1),
                )

            # =========== u_hat creation: 4 diagonal 32x32 tiles ===========
            for jc in range(0, NJ, JC):
                cps = psp.tile([128, 1536], dt.float32, tag="cps")
                xt_ch = xw.tile([128, JC * BL], dt.bfloat16, tag="xt")
                wp_ch = xw.tile([128, JC * OK], dt.bfloat16, tag="wp")
                for r in range(4):
                    nc.gpsimd.dma_start(
                        xt_ch[32 * r:32 * r + 8, :],
                        xt_d[8 * r:8 * r + 8, jc * BL:(jc + JC) * BL])
                    nc.gpsimd.dma_start(
                        wp_ch[32 * r:32 * r + 8, :],
                        wp_d[8 * r:8 * r + 8, jc * OK:(jc + JC) * OK])
                for jj in range(JC):
                    off = (jj // 3) * 512 + (jj % 3) * OK
                    for r in range(4):
                        nc.tensor.matmul(
                            cps[32 * r:32 * r + 32, off:off + OK],
                            xt_ch[32 * r:32 * r + 8, jj * BL:(jj + 1) * BL],
                            wp_ch[32 * r:32 * r + 8, jj * OK:(jj + 1) * OK],
                            start=True, stop=True,
                            tile_position=(32 * r, 32 * r),
                        )
                # drain 9 j-rounds (strided: 3 banks x 480 cols) -> bf16
                src = cps.rearrange("p (a x) -> p a x", a=3)[:, :, 0:3 * OK]
                dst = u_hat[:, jc * OK:(jc + JC) * OK].rearrange(
                    "p (a x) -> p a x", a=3)
                if (jc // JC) % 2 == 0:
                    nc.vector.tensor_copy(dst, src)
                else:
                    nc.scalar.copy(dst, src)

            # =========== iteration helpers ===========
            def squash_to_vrep(s_ps, store_out=False):
                """v = squash(s) from PSUM [32,160] (+bias);
                replicate to vrep [128,160] bf16 (or DMA out if final)."""
                s_sb = small.tile([BL, OK], dt.float32, tag="s_sb")
                nc.vector.scalar_tensor_tensor(
                    s_sb[:, :], s_ps[:, :], 0.1 if store_out is None else 1.0,
                    biasr_sb[:, :], ALU.mult, ALU.add)
                sq = small.tile([BL, OK], dt.float32, tag="sq")
                nc.scalar.activation(sq[:, :], s_sb[:, :], ACTF.Square)
                n2 = small.tile([BL, NO], dt.float32, tag="n2")
                nc.vector.tensor_reduce(
                    n2[:, :],
                    sq.rearrange("p (k o) -> p o k", o=NO),
                    AX.X, ALU.add)
                n2e = small.tile([BL, NO], dt.float32, tag="n2e")
                nc.vector.tensor_scalar_add(n2e[:, :], n2[:, :], EPS)
                sr = small.tile([BL, NO], dt.float32, tag="sr")
                nc.scalar.activation(sr[:, :], n2e[:, :], ACTF.Sqrt)
                den = small.tile([BL, NO], dt.float32, tag="den")
                nc.vector.scalar_tensor_tensor(
                    den[:, :], n2[:, :], 1.0, sr[:, :], ALU.add, ALU.mult)
                rec = small.tile([BL, NO], dt.float32, tag="rec")
                nc.vector.reciprocal(rec[:, :], den[:, :])
                g = small.tile([BL, NO], dt.float32, tag="g")
                nc.vector.tensor_mul(g[:, :], n2[:, :], rec[:, :])
                v_sb = small.tile([BL, OK], dt.float32, tag="v_sb")
                import concourse.bass as bassm
                sv = s_sb.rearrange("p (k o) -> p k o", o=NO)
                gv = g.rearrange("p (a o) -> p a o", a=1)
                sv2, gv2 = bassm.broadcast_tensor_aps(sv, gv)
                nc.vector.tensor_tensor(
                    v_sb.rearrange("p (k o) -> p k o", o=NO), sv2, gv2,
                    ALU.mult)
                if store_out:
                    nc.sync.dma_start(out_d[:, :], v_sb[:, :])
                    return
                v16 = small.tile([BL, OK], dt.bfloat16, tag="v16")
                nc.vector.tensor_copy(v16[:, :], v_sb[:, :])
                vr_ps = pss.tile([128, OK], dt.float32, tag="vr_ps")
                nc.tensor.matmul(
                    vr_ps[:, :], onesT_sb[:, :], v16[:, :],
                    start=True, stop=True)
                nc.vector.tensor_copy(vrep[:, :], vr_ps[:, :])

            def agreement_pass(first):
                """L (+)= sum_k vrep * u_hat ; per j-block on DVE."""
                for jb in range(0, NJ, JB):
                    t = tmpp.tile([128, JB * OK], dt.bfloat16, tag="t")
                    tv = t.rearrange("p (j f) -> p j f", j=JB)
                    uv = u_hat[:, jb * OK:(jb + JB) * OK].rearrange(
                        "p (j f) -> p j f", j=JB)
                    vv = vrep.rearrange("p (a f) -> p a f", a=1)
                    import concourse.bass as bassm
                    uv2, vv2 = bassm.broadcast_tensor_aps(uv, vv)
                    nc.vector.tensor_tensor(tv, uv2, vv2, ALU.mult)
                    # halving tree over k (blocks of k are stride-10 chunks)
                    kk = K
                    while kk > 1:
                        h = kk // 2
                        a0 = t.rearrange("p (j k o) -> p j k o", j=JB, k=K)
                        nc.vector.tensor_add(
                            a0[:, :, 0:h, :], a0[:, :, 0:h, :],
                            a0[:, :, h:kk, :])
                        kk = h
                    ab = t.rearrange("p (j k o) -> p j k o", j=JB, k=K)[
                        :, :, 0, :]
                    lv = L[:, jb * NO:(jb + JB) * NO].rearrange(
                        "p (j o) -> p j o", j=JB)
                    if first:
                        nc.vector.tensor_copy(lv, ab)
                    else:
                        nc.vector.tensor_add(lv, lv, ab)

            def softmax():
                nc.scalar.activation(eL[:, :], L[:, :], ACTF.Exp)
                nc.vector.tensor_reduce(
                    zr[:, :],
                    eL.rearrange("p (j o) -> p j o", o=NO),
                    AX.X, ALU.add)
                nc.vector.reciprocal(zr[:, :], zr[:, :])
                ev = eL.rearrange("p (j o) -> p j o", o=NO)
                zv = zr.rearrange("p (j a) -> p j a", a=1)
                import concourse.bass as bassm
                ev2, zv2 = bassm.broadcast_tensor_aps(ev, zv)
                nc.vector.tensor_tensor(
                    c16.rearrange("p (j o) -> p j o", o=NO), ev2, zv2,
                    ALU.mult)

            def s_pass():
                """sacc[p,(k,o)] = sum_j c*u_hat ; fold c~ via ones matmul.
                Returns s PSUM [32,160]."""
                for jb in range(0, NJ, JB):
                    t = tmpp.tile([128, JB * OK], dt.bfloat16, tag="t")
                    tv = t.rearrange("p (j k o) -> p j k o", j=JB, k=K)
                    uv = u_hat[:, jb * OK:(jb + JB) * OK].rearrange(
                        "p (j k o) -> p j k o", j=JB, k=K)
                    cv = c16[:, jb * NO:(jb + JB) * NO].rearrange(
                        "p (j a o) -> p j a o", j=JB, a=1)
                    import concourse.bass as bassm
                    uv2, cv2 = bassm.broadcast_tensor_aps(uv, cv)
                    nc.vector.tensor_tensor(tv, uv2, cv2, ALU.mult)
                    jj = JB
                    while jj > 1:
                        h = jj // 2
                        a0 = t.rearrange("p (j f) -> p j f", j=JB)
                        nc.vector.tensor_add(
                            a0[:, 0:h, :], a0[:, 0:h, :], a0[:, h:jj, :])
                        jj = h
                    blk = t[:, 0:OK]
                    if jb == 0:
                        nc.vector.tensor_copy(sacc[:, :], blk)
                    else:
                        nc.vector.tensor_add(sacc[:, :], sacc[:, :], blk)
                nc.vector.tensor_copy(sacc16[:, :], sacc[:, :])
                s_ps = pss.tile([BL, OK], dt.float32, tag="sps")
                nc.tensor.matmul(
                    s_ps[:, :], ones_sb[:, :], sacc16[:, :],
                    start=True, stop=True)
                return s_ps

            # =========== routing ===========
            tc.strict_bb_all_engine_barrier()
            # pre-observe const DMA queues on DVE/ACT so later ops need <=1 wait
            nc.vector.tensor_copy(touch[:, 0:1], biasr_sb[0:1, 0:1])
            nc.scalar.copy(touch[:, 1:2], biasr_sb[0:1, 1:2])
            # iter 1: c uniform=0.1 -> v1 from s1 (scale 0.1 applied in squash)
            squash_to_vrep(s1_ps, store_out=None)   # store_out=None => scale .1
            if debug:
                nc.sync.dma_start(dbg_u[:, :], u_hat[:, :])
                nc.sync.dma_start(dbg_v1[:, :], vrep[:, :])
            tc.strict_bb_all_engine_barrier()
            agreement_pass(first=True)              # L = a1
            if debug:
                nc.sync.dma_start(dbg_L[:, :], L[:, :])
            tc.strict_bb_all_engine_barrier()
            softmax()
            s2 = s_pass()
            squash_to_vrep(s2)                      # v2 -> vrep
            tc.strict_bb_all_engine_barrier()
            agreement_pass(first=False)             # L += a2
            softmax()
            tc.strict_bb_all_engine_barrier()
            s3 = s_pass()
            squash_to_vrep(s3, store_out=True)      # final v -> DRAM

    nc.finalize()
    return nc


def kernel(x, W, bias):
    x = np.asarray(x, dtype=np.float32)
    W = np.asarray(W, dtype=np.float32)
    bias = np.asarray(bias, dtype=np.float32)

    from concourse.bass_utils import run_bass_kernel_spmd

    if "nc" not in _CACHE:
        _CACHE["nc"] = _build_bass()
    nc = _CACHE["nc"]

    in_maps = _pack_inputs(x, W, bias)
    res = run_bass_kernel_spmd(nc, in_maps, core_ids=list(range(NC)))
    _CACHE["last_results"] = res

    out = np.zeros((B, NO, K), dtype=np.float32)
    for cid in range(NC):
        v = res.results[cid]["out_v"]          # [32, 160] in (k,o) order
        out[cid * BL:(cid + 1) * BL] = (
            v.reshape(BL, K, NO).transpose(0, 2, 1))
    return out


if __name__ == "__main__":
    import reference
    inputs = reference.setup_inputs()
    inputs = {k: np.asarray(v) for k, v in inputs.items()}
    expected = np.asarray(reference.reference(**inputs))
    actual = kernel(**inputs)
    err = np.abs(actual - expected).max() / (np.abs(expected).max() + 1e-12)
    print("Relative error:", err)


# revision 19
# speedup vs baseline: 1.1755x; 1.1755x over previous
"""DigitCaps dynamic-routing kernel for 8 Trainium2 NeuronCores.

Strategy (batch-sharded, fully local per core, no collectives):
  B=256 -> 8 cores x 32 batch rows. Each core computes u_hat for its batch
  shard with the PE (4 diagonal 32x32 tiles per round: i%4 -> row/col group,
  contraction d=8), keeps u_hat resident in SBUF as bf16 in layout
      u_hat[p = 32*(i%4) + b, free = (i//4)*160 + k*10 + o]
  and runs the 3 routing iterations on-chip:
    iter1: c uniform -> s1 = 0.1*sum_i u_hat + bias via a dense PE matmul
           over the joint (i,d)=9216 contraction (host-packed dense layouts).
    agreement passes: DVE bf16 2x multiply (V broadcast) + halving-tree sum
           over k; logits/softmax per (b,i) are partition-local.
    s passes: DVE multiply by c (broadcast over k) + halving tree over i//4,
           then a ones-block-diagonal PE matmul folds the 4 i%4 partition
           groups; squash is computed on [32,160] tiles.
  Host pre-packs x/W into the exact SBUF images (transpose+pad in numpy) so
  every DMA is a plain contiguous copy.
"""

import numpy as np

B, NI, DI, NO, K = 256, 1152, 8, 10, 16
NC = 8
BL = B // NC            # 32 batch rows per core
NJ = NI // 4            # 288 j-rounds (i = 4*j + r)
OK = NO * K             # 160, stored in (k, o) order: idx = k*10 + o
NQ = NI // 16           # 72 dense chunks (i = 16*q + t)
EPS = 1e-9

# creation PSUM chunking: 9 j-rounds per psum tile, 3 j per 512-col bank slot
JC = 9
# DVE block size (j per block) for routing passes
JB = 32

_CACHE = {}


def _pack_inputs(x, W, bias):
    """Host-side packing into per-core DMA images (all plain contiguous)."""
    import ml_dtypes
    bf16 = ml_dtypes.bfloat16

    # xt (padded transpose, creation lhsT): [32, NJ*32] rows=(r,d) dense 32
    #   xt[8*r + d, j*32 + b] = x[b0+b, 4*j+r, d]
    # stored dense [32 rows, NJ*BL]; DMA'd per r-group into partitions 32r..32r+8
    xr = x.reshape(B, NJ, 4, DI)                       # [b, j, r, d]
    xt_all = np.ascontiguousarray(
        xr.transpose(2, 3, 1, 0)).astype(bf16)         # [r, d, j, b_all]

    # dense xtD (s1 lhsT): [128, NQ*32]: xtD[16*?] rows = (t, d) = 128
    xd = x.reshape(B, NQ, 16, DI)                      # [b, q, t, d]
    xtD_all = np.ascontiguousarray(
        xd.transpose(2, 3, 1, 0)).astype(bf16)         # [t, d, q, b_all]

    # wp (creation rhs stream): dense rows [32, NJ*OK]
    #   wp[8*r + d, j*160 + k*10 + o] = W[4*j+r, o, k, d]
    wr = W.reshape(NJ, 4, NO, K, DI)                   # [j, r, o, k, d]
    wp = np.ascontiguousarray(
        wr.transpose(1, 4, 0, 3, 2).reshape(32, NJ * OK)).astype(bf16)

    # wd (s1 rhs stream, dense): [128, NQ*OK]
    #   wd[8*t + d, q*160 + k*10 + o] = W[16*q+t, o, k, d]
    wq = W.reshape(NQ, 16, NO, K, DI)                  # [q, t, o, k, d]
    wd = np.ascontiguousarray(
        wq.transpose(1, 4, 0, 3, 2).reshape(128, NQ * OK)).astype(bf16)

    # ones block-diag for folding 4 partition groups: [128, 32]
    ones_bd = np.zeros((128, BL), dtype=np.float32)
    for c in range(4):
        ones_bd[np.arange(BL) + 32 * c, np.arange(BL)] = 1.0
    ones_bd = ones_bd.astype(bf16)

    # onesT for partition replication via PE: [32, 128]
    onesT = np.zeros((BL, 128), dtype=np.float32)
    for c in range(4):
        onesT[np.arange(BL), np.arange(BL) + 32 * c] = 1.0
    onesT = onesT.astype(bf16)

    # bias replicated [32, 160] f32 in (k, o) order
    biasr = np.ascontiguousarray(
        np.broadcast_to(bias.T.reshape(1, OK), (BL, OK))).astype(np.float32)

    per_core = []
    for cid in range(NC):
        b0 = cid * BL
        xt = np.ascontiguousarray(
            xt_all[:, :, :, b0:b0 + BL].reshape(32, NJ * BL))
        xtD = np.ascontiguousarray(
            xtD_all[:, :, :, b0:b0 + BL].reshape(128, NQ * BL))
        per_core.append({
            "xt": xt, "xtd": xtD, "wp": wp, "wd": wd,
            "ones_bd": ones_bd, "onesT": onesT, "biasr": biasr,
        })
    return per_core


def _build_bass(debug=False, upto=99):
    import concourse.bass as bass
    import concourse.bacc as bacc
    import concourse.mybir as mybir
    from concourse.tile import TileContext

    dt = mybir.dt
    ALU = mybir.AluOpType
    ACTF = mybir.ActivationFunctionType
    AX = mybir.AxisListType

    nc = bacc.Bacc()

    xt_d = nc.dram_tensor("xt", [32, NJ * BL], dt.bfloat16, kind="ExternalInput")
    xtD_d = nc.dram_tensor("xtd", [128, NQ * BL], dt.bfloat16, kind="ExternalInput")
    wp_d = nc.dram_tensor("wp", [32, NJ * OK], dt.bfloat16, kind="ExternalInput")
    wd_d = nc.dram_tensor("wd", [128, NQ * OK], dt.bfloat16, kind="ExternalInput")
    ones_d = nc.dram_tensor("ones_bd", [128, BL], dt.bfloat16, kind="ExternalInput")
    onesT_d = nc.dram_tensor("onesT", [BL, 128], dt.bfloat16, kind="ExternalInput")
    biasr_d = nc.dram_tensor("biasr", [BL, OK], dt.float32, kind="ExternalInput")
    out_d = nc.dram_tensor("out_v", [BL, OK], dt.float32, kind="ExternalOutput")
    if debug:
        dbg_u = nc.dram_tensor("dbg_u", [128, NJ * OK], dt.bfloat16, kind="ExternalOutput")
        dbg_L = nc.dram_tensor("dbg_L", [128, NJ * NO], dt.float32, kind="ExternalOutput")
        dbg_v1 = nc.dram_tensor("dbg_v1", [128, OK], dt.bfloat16, kind="ExternalOutput")


    with TileContext(nc) as tc:
        with (
            tc.tile_pool(name="const", bufs=1) as const,
            tc.tile_pool(name="big", bufs=1) as big,
            tc.tile_pool(name="xw", bufs=4) as xw,
            tc.tile_pool(name="tmp", bufs=2) as tmpp,
            tc.tile_pool(name="small", bufs=2) as small,
            tc.tile_pool(name="ps", bufs=2, space="PSUM") as psp,
            tc.tile_pool(name="pss", bufs=1, space="PSUM") as pss,
        ):
            # ---- resident tiles ----
            u_hat = big.tile([128, NJ * OK], dt.bfloat16)     # 92KB/part
            L = big.tile([128, NJ * NO], dt.float32)          # logits [p,(j,o)]
            eL = big.tile([128, NJ * NO], dt.float32)         # exp(L)
            c16 = big.tile([128, NJ * NO], dt.bfloat16)       # softmax out
            xtD_sb = const.tile([128, NQ * BL], dt.bfloat16)
            ones_sb = const.tile([128, BL], dt.bfloat16)
            onesT_sb = const.tile([BL, 128], dt.bfloat16)
            biasr_sb = const.tile([BL, OK], dt.float32)
            vrep = const.tile([128, OK], dt.bfloat16)         # V replicated x4
            sacc = const.tile([128, OK], dt.float32)          # s partial (c~,b)
            sacc16 = const.tile([128, OK], dt.bfloat16)
            zr = const.tile([128, NJ], dt.float32)

            nc.sync.dma_start(xtD_sb[:, :], xtD_d[:, :])
            touch = const.tile([1, 4], dt.float32)
            nc.sync.dma_start(ones_sb[:, :], ones_d[:, :])
            nc.sync.dma_start(onesT_sb[:, :], onesT_d[:, :])
            nc.sync.dma_start(biasr_sb[:, :], biasr_d[:, :])

            # =========== s1: dense (i,d) contraction ===========
            wd_sb = const.tile([128, NQ * OK], dt.bfloat16)
            nc.sync.dma_start(wd_sb[:, :], wd_d[:, :])
            s1_ps = pss.tile([BL, OK], dt.float32, tag="sps")
            for q in range(NQ):
                nc.tensor.matmul(
                    s1_ps[:, :],
                    xtD_sb[:, q * BL:(q + 1) * BL],
                    wd_sb[:, q * OK:(q + 1) * OK],
                    start=(q == 0), stop=(q == NQ - 1),
                )

            # =========== u_hat creation: 4 diagonal 32x32 tiles ===========
            for jc in range(0, NJ, JC):
                cps = psp.tile([128, 1536], dt.float32, tag="cps")
                xt_ch = xw.tile([128, JC * BL], dt.bfloat16, tag="xt")
                wp_ch = xw.tile([128, JC * OK], dt.bfloat16, tag="wp")
                for r in range(4):
                    nc.gpsimd.dma_start(
                        xt_ch[32 * r:32 * r + 8, :],
                        xt_d[8 * r:8 * r + 8, jc * BL:(jc + JC) * BL])
                    nc.gpsimd.dma_start(
                        wp_ch[32 * r:32 * r + 8, :],
                        wp_d[8 * r:8 * r + 8, jc * OK:(jc + JC) * OK])
                for jj in range(JC):
                    off = (jj // 3) * 512 + (jj % 3) * OK
                    for r in range(4):
                        nc.tensor.matmul(
                            cps[32 * r:32 * r + 32, off:off + OK],
                            xt_ch[32 * r:32 * r + 8, jj * BL:(jj + 1) * BL],
                            wp_ch[32 * r:32 * r + 8, jj * OK:(jj + 1) * OK],
                            start=True, stop=True,
                            tile_position=(32 * r, 32 * r),
                        )
                # drain 9 j-rounds (strided: 3 banks x 480 cols) -> bf16
                src = cps.rearrange("p (a x) -> p a x", a=3)[:, :, 0:3 * OK]
                dst = u_hat[:, jc * OK:(jc + JC) * OK].rearrange(
                    "p (a x) -> p a x", a=3)
                if (jc // JC) % 2 == 0:
                    nc.vector.tensor_copy(dst, src)
                else:
                    nc.scalar.copy(dst, src)

            # =========== iteration helpers ===========
            def squash_to_vrep(s_ps, store_out=False):
                """v = squash(s) from PSUM [32,160] (+bias);
                replicate to vrep [128,160] bf16 (or DMA out if final)."""
                s_sb = small.tile([BL, OK], dt.float32, tag="s_sb")
                nc.vector.scalar_tensor_tensor(
                    s_sb[:, :], s_ps[:, :], 0.1 if store_out is None else 1.0,
                    biasr_sb[:, :], ALU.mult, ALU.add)
                sq = small.tile([BL, OK], dt.float32, tag="sq")
                nc.scalar.activation(sq[:, :], s_sb[:, :], ACTF.Square)
                n2 = small.tile([BL, NO], dt.float32, tag="n2")
                nc.vector.tensor_reduce(
                    n2[:, :],
                    sq.rearrange("p (k o) -> p o k", o=NO),
                    AX.X, ALU.add)
                n2e = small.tile([BL, NO], dt.float32, tag="n2e")
                nc.vector.tensor_scalar_add(n2e[:, :], n2[:, :], EPS)
                sr = small.tile([BL, NO], dt.float32, tag="sr")
                nc.scalar.activation(sr[:, :], n2e[:, :], ACTF.Sqrt)
                den = small.tile([BL, NO], dt.float32, tag="den")
                nc.vector.scalar_tensor_tensor(
                    den[:, :], n2[:, :], 1.0, sr[:, :], ALU.add, ALU.mult)
                rec = small.tile([BL, NO], dt.float32, tag="rec")
                nc.vector.reciprocal(rec[:, :], den[:, :])
                g = small.tile([BL, NO], dt.float32, tag="g")
                nc.vector.tensor_mul(g[:, :], n2[:, :], rec[:, :])
                v_sb = small.tile([BL, OK], dt.float32, tag="v_sb")
                import concourse.bass as bassm
                sv = s_sb.rearrange("p (k o) -> p k o", o=NO)
                gv = g.rearrange("p (a o) -> p a o", a=1)
                sv2, gv2 = bassm.broadcast_tensor_aps(sv, gv)
                nc.vector.tensor_tensor(
                    v_sb.rearrange("p (k o) -> p k o", o=NO), sv2, gv2,
                    ALU.mult)
                if store_out:
                    nc.sync.dma_start(out_d[:, :], v_sb[:, :])
                    return
                v16 = small.tile([BL, OK], dt.bfloat16, tag="v16")
                nc.vector.tensor_copy(v16[:, :], v_sb[:, :])
                vr_ps = pss.tile([128, OK], dt.float32, tag="vr_ps")
                nc.tensor.matmul(
                    vr_ps[:, :], onesT_sb[:, :], v16[:, :],
                    start=True, stop=True)
                nc.vector.tensor_copy(vrep[:, :], vr_ps[:, :])

            def agreement_pass(first):
                """L (+)= sum_k vrep * u_hat ; per j-block on DVE."""
                for jb in range(0, NJ, JB):
                    t = tmpp.tile([128, JB * OK], dt.bfloat16, tag="t")
                    tv = t.rearrange("p (j f) -> p j f", j=JB)
                    uv = u_hat[:, jb * OK:(jb + JB) * OK].rearrange(
                        "p (j f) -> p j f", j=JB)
                    vv = vrep.rearrange("p (a f) -> p a f", a=1)
                    import concourse.bass as bassm
                    uv2, vv2 = bassm.broadcast_tensor_aps(uv, vv)
                    nc.vector.tensor_tensor(tv, uv2, vv2, ALU.mult)
                    # halving tree over k (blocks of k are stride-10 chunks)
                    kk = K
                    while kk > 1:
                        h = kk // 2
                        a0 = t.rearrange("p (j k o) -> p j k o", j=JB, k=K)
                        nc.vector.tensor_add(
                            a0[:, :, 0:h, :], a0[:, :, 0:h, :],
                            a0[:, :, h:kk, :])
                        kk = h
                    ab = t.rearrange("p (j k o) -> p j k o", j=JB, k=K)[
                        :, :, 0, :]
                    lv = L[:, jb * NO:(jb + JB) * NO].rearrange(
                        "p (j o) -> p j o", j=JB)
                    if first:
                        nc.vector.tensor_copy(lv, ab)
                    else:
                        nc.vector.tensor_add(lv, lv, ab)

            def softmax():
                nc.scalar.activation(eL[:, :], L[:, :], ACTF.Exp)
                nc.vector.tensor_reduce(
                    zr[:, :],
                    eL.rearrange("p (j o) -> p j o", o=NO),
                    AX.X, ALU.add)
                nc.vector.reciprocal(zr[:, :], zr[:, :])
                ev = eL.rearrange("p (j o) -> p j o", o=NO)
                zv = zr.rearrange("p (j a) -> p j a", a=1)
                import concourse.bass as bassm
                ev2, zv2 = bassm.broadcast_tensor_aps(ev, zv)
                nc.vector.tensor_tensor(
                    c16.rearrange("p (j o) -> p j o", o=NO), ev2, zv2,
                    ALU.mult)

            def s_pass():
                """sacc[p,(k,o)] = sum_j c*u_hat ; fold c~ via ones matmul.
                Returns s PSUM [32,160]."""
                for jb in range(0, NJ, JB):
                    t = tmpp.tile([128, JB * OK], dt.bfloat16, tag="t")
                    tv = t.rearrange("p (j k o) -> p j k o", j=JB, k=K)
                    uv = u_hat[:, jb * OK:(jb + JB) * OK].rearrange(
                        "p (j k o) -> p j k o", j=JB, k=K)
                    cv = c16[:, jb * NO:(jb + JB) * NO].rearrange(
                        "p (j a o) -> p j a o", j=JB, a=1)
                    import concourse.bass as bassm
                    uv2, cv2 = bassm.broadcast_tensor_aps(uv, cv)
                    nc.vector.tensor_tensor(tv, uv2, cv2, ALU.mult)
                    jj = JB
                    while jj > 1:
                        h = jj // 2
                        a0 = t.rearrange("p (j f) -> p j f", j=JB)
                        nc.vector.tensor_add(
                            a0[:, 0:h, :], a0[:, 0:h, :], a0[:, h:jj, :])
                        jj = h
                    blk = t[:, 0:OK]
                    if jb == 0:
                        nc.vector.tensor_copy(sacc[:, :], blk)
                    else:
                        nc.vector.tensor_add(sacc[:, :], sacc[:, :], blk)
                nc.vector.tensor_copy(sacc16[:, :], sacc[:, :])
                s_ps = pss.tile([BL, OK], dt.float32, tag="sps")
                nc.tensor.matmul(
                    s_ps[:, :], ones_sb[:, :], sacc16[:, :],
                    start=True, stop=True)
                return s_ps

            # =========== routing ===========
            done = [False]

            def stop_here():
                nc.sync.dma_start(out_d[:, :], biasr_sb[:, :])
                done[0] = True

            if upto <= 0:
                stop_here()
            else:
                tc.strict_bb_all_engine_barrier()
            # pre-observe const DMA queues on DVE/ACT so later ops need <=1 wait
            nc.vector.tensor_copy(touch[:, 0:1], biasr_sb[0:1, 0:1])
            nc.scalar.copy(touch[:, 1:2], biasr_sb[0:1, 1:2])
            # iter 1: c uniform=0.1 -> v1 from s1 (scale 0.1 applied in squash)
            squash_to_vrep(s1_ps, store_out=None)   # store_out=None => scale .1
            if debug:
                nc.sync.dma_start(dbg_u[:, :], u_hat[:, :])
                nc.sync.dma_start(dbg_v1[:, :], vrep[:, :])
            tc.strict_bb_all_engine_barrier()
            if not done[0] and upto <= 1:
                stop_here()
            if not done[0]:
                agreement_pass(first=True)          # L = a1
            if debug:
                nc.sync.dma_start(dbg_L[:, :], L[:, :])
            tc.strict_bb_all_engine_barrier()
            if not done[0] and upto <= 2:
                stop_here()
            if not done[0]:
                softmax()
                s2 = s_pass()
                squash_to_vrep(s2)                  # v2 -> vrep
            if not done[0]:
                tc.strict_bb_all_engine_barrier()
            if not done[0] and upto <= 3:
                stop_here()
            if not done[0]:
                agreement_pass(first=False)         # L += a2
            if not done[0]:
                softmax()
                tc.strict_bb_all_engine_barrier()
                s3 = s_pass()
                squash_to_vrep(s3, store_out=True)  # final v -> DRAM

    nc.finalize()
    return nc


def kernel(x, W, bias):
    x = np.asarray(x, dtype=np.float32)
    W = np.asarray(W, dtype=np.float32)
    bias = np.asarray(bias, dtype=np.float32)

    from concourse.bass_utils import run_bass_kernel_spmd

    if "nc" not in _CACHE:
        _CACHE["nc"] = _build_bass()
    nc = _CACHE["nc"]

    in_maps = _pack_inputs(x, W, bias)
    res = run_bass_kernel_spmd(nc, in_maps, core_ids=list(range(NC)))
    _CACHE["last_results"] = res

    out = np.zeros((B, NO, K), dtype=np.float32)
    for cid in range(NC):
        v = res.results[cid]["out_v"]          # [32, 160] in (k,o) order
        out[cid * BL:(cid + 1) * BL] = (
            v.reshape(BL, K, NO).transpose(0, 2, 1))
    return out


if __name__ == "__main__":
    import reference
    inputs = reference.setup_inputs()
    inputs = {k: np.asarray(v) for k, v in inputs.items()}
    expected = np.asarray(reference.reference(**inputs))
    actual = kernel(**inputs)
    err = np.abs(actual - expected).max() / (np.abs(expected).max() + 1e-12)
    print("Relative error:", err)


# revision 20
# speedup vs baseline: 2302.6407x; 1958.8756x over previous
"""DigitCaps dynamic-routing kernel for 8 Trainium2 NeuronCores.

Strategy (batch-sharded, fully local per core, no collectives):
  B=256 -> 8 cores x 32 batch rows. Each core computes u_hat for its batch
  shard with the PE (4 diagonal 32x32 tiles per round: i%4 -> row/col group,
  contraction d=8), keeps u_hat resident in SBUF as bf16 in layout
      u_hat[p = 32*(i%4) + b, free = (i//4)*160 + k*10 + o]
  and runs the 3 routing iterations on-chip:
    iter1: c uniform -> s1 = 0.1*sum_i u_hat + bias via a dense PE matmul
           over the joint (i,d)=9216 contraction (host-packed dense layouts).
    agreement passes: DVE bf16 2x multiply (V broadcast) + halving-tree sum
           over k; logits/softmax per (b,i) are partition-local.
    s passes: DVE multiply by c (broadcast over k) + halving tree over i//4,
           then a ones-block-diagonal PE matmul folds the 4 i%4 partition
           groups; squash is computed on [32,160] tiles.
  Host pre-packs x/W into the exact SBUF images (transpose+pad in numpy) so
  every DMA is a plain contiguous copy.
"""

import numpy as np

B, NI, DI, NO, K = 256, 1152, 8, 10, 16
NC = 8
BL = B // NC            # 32 batch rows per core
NJ = NI // 4            # 288 j-rounds (i = 4*j + r)
OK = NO * K             # 160, stored in (k, o) order: idx = k*10 + o
NQ = NI // 16           # 72 dense chunks (i = 16*q + t)
EPS = 1e-9

# creation PSUM chunking: 9 j-rounds per psum tile, 3 j per 512-col bank slot
JC = 9
# DVE block size (j per block) for routing passes
JB = 32

_CACHE = {}


def _pack_inputs(x, W, bias):
    """Host-side packing into per-core DMA images (all plain contiguous)."""
    import ml_dtypes
    bf16 = ml_dtypes.bfloat16

    # xt (padded transpose, creation lhsT): [32, NJ*32] rows=(r,d) dense 32
    #   xt[8*r + d, j*32 + b] = x[b0+b, 4*j+r, d]
    # stored dense [32 rows, NJ*BL]; DMA'd per r-group into partitions 32r..32r+8
    xr = x.reshape(B, NJ, 4, DI)                       # [b, j, r, d]
    xt_all = np.ascontiguousarray(
        xr.transpose(2, 3, 1, 0)).astype(bf16)         # [r, d, j, b_all]

    # dense xtD (s1 lhsT): [128, NQ*32]: xtD[16*?] rows = (t, d) = 128
    xd = x.reshape(B, NQ, 16, DI)                      # [b, q, t, d]
    xtD_all = np.ascontiguousarray(
        xd.transpose(2, 3, 1, 0)).astype(bf16)         # [t, d, q, b_all]

    # wp (creation rhs stream): dense rows [32, NJ*OK]
    #   wp[8*r + d, j*160 + k*10 + o] = W[4*j+r, o, k, d]
    wr = W.reshape(NJ, 4, NO, K, DI)                   # [j, r, o, k, d]
    wp = np.ascontiguousarray(
        wr.transpose(1, 4, 0, 3, 2).reshape(32, NJ * OK)).astype(bf16)

    # wd (s1 rhs stream, dense): [128, NQ*OK]
    #   wd[8*t + d, q*160 + k*10 + o] = W[16*q+t, o, k, d]
    wq = W.reshape(NQ, 16, NO, K, DI)                  # [q, t, o, k, d]
    wd = np.ascontiguousarray(
        wq.transpose(1, 4, 0, 3, 2).reshape(128, NQ * OK)).astype(bf16)

    # ones block-diag for folding 4 partition groups: [128, 32]
    ones_bd = np.zeros((128, BL), dtype=np.float32)
    for c in range(4):
        ones_bd[np.arange(BL) + 32 * c, np.arange(BL)] = 1.0
    ones_bd = ones_bd.astype(bf16)

    # onesT for partition replication via PE: [32, 128]
    onesT = np.zeros((BL, 128), dtype=np.float32)
    for c in range(4):
        onesT[np.arange(BL), np.arange(BL) + 32 * c] = 1.0
    onesT = onesT.astype(bf16)

    # bias replicated [32, 160] f32 in (k, o) order
    biasr = np.ascontiguousarray(
        np.broadcast_to(bias.T.reshape(1, OK), (BL, OK))).astype(np.float32)

    per_core = []
    for cid in range(NC):
        b0 = cid * BL
        xt = np.ascontiguousarray(
            xt_all[:, :, :, b0:b0 + BL].reshape(32, NJ * BL))
        xtD = np.ascontiguousarray(
            xtD_all[:, :, :, b0:b0 + BL].reshape(128, NQ * BL))
        per_core.append({
            "xt": xt, "xtd": xtD, "wp": wp, "wd": wd,
            "ones_bd": ones_bd, "onesT": onesT, "biasr": biasr,
        })
    return per_core


def _build_bass(debug=False, upto=99):
    import concourse.bass as bass
    import concourse.bacc as bacc
    import concourse.mybir as mybir
    from concourse.tile import TileContext

    dt = mybir.dt
    ALU = mybir.AluOpType
    ACTF = mybir.ActivationFunctionType
    AX = mybir.AxisListType

    nc = bacc.Bacc()

    xt_d = nc.dram_tensor("xt", [32, NJ * BL], dt.bfloat16, kind="ExternalInput")
    xtD_d = nc.dram_tensor("xtd", [128, NQ * BL], dt.bfloat16, kind="ExternalInput")
    wp_d = nc.dram_tensor("wp", [32, NJ * OK], dt.bfloat16, kind="ExternalInput")
    wd_d = nc.dram_tensor("wd", [128, NQ * OK], dt.bfloat16, kind="ExternalInput")
    ones_d = nc.dram_tensor("ones_bd", [128, BL], dt.bfloat16, kind="ExternalInput")
    onesT_d = nc.dram_tensor("onesT", [BL, 128], dt.bfloat16, kind="ExternalInput")
    biasr_d = nc.dram_tensor("biasr", [BL, OK], dt.float32, kind="ExternalInput")
    out_d = nc.dram_tensor("out_v", [BL, OK], dt.float32, kind="ExternalOutput")
    if debug:
        dbg_u = nc.dram_tensor("dbg_u", [128, NJ * OK], dt.bfloat16, kind="ExternalOutput")
        dbg_L = nc.dram_tensor("dbg_L", [128, NJ * NO], dt.float32, kind="ExternalOutput")
        dbg_v1 = nc.dram_tensor("dbg_v1", [128, OK], dt.bfloat16, kind="ExternalOutput")


    with TileContext(nc) as tc:
        with (
            tc.tile_pool(name="const", bufs=1) as const,
            tc.tile_pool(name="big", bufs=1) as big,
            tc.tile_pool(name="xw", bufs=4) as xw,
            tc.tile_pool(name="tmp", bufs=3) as tmpp,
            tc.tile_pool(name="small", bufs=2) as small,
            tc.tile_pool(name="ps", bufs=2, space="PSUM") as psp,
            tc.tile_pool(name="pss", bufs=1, space="PSUM") as pss,
        ):
            # ---- resident tiles ----
            u_hat = big.tile([128, NJ * OK], dt.bfloat16)     # 92KB/part
            L = big.tile([128, NJ * NO], dt.float32)          # logits [p,(j,o)]
            eL = big.tile([128, NJ * NO], dt.float32)         # exp(L)
            c16 = big.tile([128, NJ * NO], dt.bfloat16)       # softmax out
            xtD_sb = const.tile([128, NQ * BL], dt.bfloat16)
            ones_sb = const.tile([128, BL], dt.bfloat16)
            onesT_sb = const.tile([BL, 128], dt.bfloat16)
            biasr_sb = const.tile([BL, OK], dt.float32)
            vrep = const.tile([128, OK], dt.bfloat16)         # V replicated x4
            sacc = const.tile([128, OK], dt.float32)          # s partial (c~,b)
            sacc16 = const.tile([128, OK], dt.bfloat16)
            zr = const.tile([128, NJ], dt.float32)

            nc.sync.dma_start(xtD_sb[:, :], xtD_d[:, :])
            touch = const.tile([1, 4], dt.float32)
            nc.sync.dma_start(ones_sb[:, :], ones_d[:, :])
            nc.sync.dma_start(onesT_sb[:, :], onesT_d[:, :])
            nc.sync.dma_start(biasr_sb[:, :], biasr_d[:, :])

            # =========== s1: dense (i,d) contraction ===========
            wd_sb = const.tile([128, NQ * OK], dt.bfloat16)
            nc.sync.dma_start(wd_sb[:, :], wd_d[:, :])
            s1_ps = pss.tile([BL, OK], dt.float32, tag="sps")
            for q in range(NQ):
                nc.tensor.matmul(
                    s1_ps[:, :],
                    xtD_sb[:, q * BL:(q + 1) * BL],
                    wd_sb[:, q * OK:(q + 1) * OK],
                    start=(q == 0), stop=(q == NQ - 1),
                )

            # =========== u_hat creation: 4 diagonal 32x32 tiles ===========
            for jc in range(0, NJ, JC):
                cps = psp.tile([128, 1536], dt.float32, tag="cps")
                xt_ch = xw.tile([128, JC * BL], dt.bfloat16, tag="xt")
                wp_ch = xw.tile([128, JC * OK], dt.bfloat16, tag="wp")
                for r in range(4):
                    nc.gpsimd.dma_start(
                        xt_ch[32 * r:32 * r + 8, :],
                        xt_d[8 * r:8 * r + 8, jc * BL:(jc + JC) * BL])
                    nc.gpsimd.dma_start(
                        wp_ch[32 * r:32 * r + 8, :],
                        wp_d[8 * r:8 * r + 8, jc * OK:(jc + JC) * OK])
                for jj in range(JC):
                    off = (jj // 3) * 512 + (jj % 3) * OK
                    for r in range(4):
                        nc.tensor.matmul(
                            cps[32 * r:32 * r + 32, off:off + OK],
                            xt_ch[32 * r:32 * r + 8, jj * BL:(jj + 1) * BL],
                            wp_ch[32 * r:32 * r + 8, jj * OK:(jj + 1) * OK],
                            start=True, stop=True,
                            tile_position=(32 * r, 32 * r),
                        )
                # drain 9 j-rounds (strided: 3 banks x 480 cols) -> bf16
                src = cps.rearrange("p (a x) -> p a x", a=3)[:, :, 0:3 * OK]
                dst = u_hat[:, jc * OK:(jc + JC) * OK].rearrange(
                    "p (a x) -> p a x", a=3)
                if (jc // JC) % 2 == 0:
                    nc.vector.tensor_copy(dst, src)
                else:
                    nc.scalar.copy(dst, src)

            # =========== iteration helpers ===========
            def squash_to_vrep(s_ps, store_out=False):
                """v = squash(s) from PSUM [32,160] (+bias);
                replicate to vrep [128,160] bf16 (or DMA out if final)."""
                s_sb = small.tile([BL, OK], dt.float32, tag="s_sb")
                nc.vector.scalar_tensor_tensor(
                    s_sb[:, :], s_ps[:, :], 0.1 if store_out is None else 1.0,
                    biasr_sb[:, :], ALU.mult, ALU.add)
                sq = small.tile([BL, OK], dt.float32, tag="sq")
                nc.scalar.activation(sq[:, :], s_sb[:, :], ACTF.Square)
                n2 = small.tile([BL, NO], dt.float32, tag="n2")
                nc.vector.tensor_reduce(
                    n2[:, :],
                    sq.rearrange("p (k o) -> p o k", o=NO),
                    AX.X, ALU.add)
                n2e = small.tile([BL, NO], dt.float32, tag="n2e")
                nc.vector.tensor_scalar_add(n2e[:, :], n2[:, :], EPS)
                sr = small.tile([BL, NO], dt.float32, tag="sr")
                nc.scalar.activation(sr[:, :], n2e[:, :], ACTF.Sqrt)
                den = small.tile([BL, NO], dt.float32, tag="den")
                nc.vector.scalar_tensor_tensor(
                    den[:, :], n2[:, :], 1.0, sr[:, :], ALU.add, ALU.mult)
                rec = small.tile([BL, NO], dt.float32, tag="rec")
                nc.vector.reciprocal(rec[:, :], den[:, :])
                g = small.tile([BL, NO], dt.float32, tag="g")
                nc.vector.tensor_mul(g[:, :], n2[:, :], rec[:, :])
                v_sb = small.tile([BL, OK], dt.float32, tag="v_sb")
                import concourse.bass as bassm
                sv = s_sb.rearrange("p (k o) -> p k o", o=NO)
                gv = g.rearrange("p (a o) -> p a o", a=1)
                sv2, gv2 = bassm.broadcast_tensor_aps(sv, gv)
                nc.vector.tensor_tensor(
                    v_sb.rearrange("p (k o) -> p k o", o=NO), sv2, gv2,
                    ALU.mult)
                if store_out:
                    nc.sync.dma_start(out_d[:, :], v_sb[:, :])
                    return
                v16 = small.tile([BL, OK], dt.bfloat16, tag="v16")
                nc.vector.tensor_copy(v16[:, :], v_sb[:, :])
                vr_ps = pss.tile([128, OK], dt.float32, tag="vr_ps")
                nc.tensor.matmul(
                    vr_ps[:, :], onesT_sb[:, :], v16[:, :],
                    start=True, stop=True)
                nc.vector.tensor_copy(vrep[:, :], vr_ps[:, :])

            def agreement_pass(first):
                """L (+)= sum_k vrep * u_hat ; per j-block on DVE."""
                for jb in range(0, NJ, JB):
                    t = tmpp.tile([128, JB * OK], dt.bfloat16, tag="t")
                    tv = t.rearrange("p (j f) -> p j f", j=JB)
                    uv = u_hat[:, jb * OK:(jb + JB) * OK].rearrange(
                        "p (j f) -> p j f", j=JB)
                    vv = vrep.rearrange("p (a f) -> p a f", a=1)
                    import concourse.bass as bassm
                    uv2, vv2 = bassm.broadcast_tensor_aps(uv, vv)
                    nc.vector.tensor_tensor(tv, uv2, vv2, ALU.mult)
                    # halving tree over k (blocks of k are stride-10 chunks)
                    kk = K
                    while kk > 1:
                        h = kk // 2
                        a0 = t.rearrange("p (j k o) -> p j k o", j=JB, k=K)
                        nc.vector.tensor_add(
                            a0[:, :, 0:h, :], a0[:, :, 0:h, :],
                            a0[:, :, h:kk, :])
                        kk = h
                    ab = t.rearrange("p (j k o) -> p j k o", j=JB, k=K)[
                        :, :, 0, :]
                    lv = L[:, jb * NO:(jb + JB) * NO].rearrange(
                        "p (j o) -> p j o", j=JB)
                    if first:
                        nc.vector.tensor_copy(lv, ab)
                    else:
                        nc.vector.tensor_add(lv, lv, ab)

            def softmax():
                nc.scalar.activation(eL[:, :], L[:, :], ACTF.Exp)
                nc.vector.tensor_reduce(
                    zr[:, :],
                    eL.rearrange("p (j o) -> p j o", o=NO),
                    AX.X, ALU.add)
                nc.vector.reciprocal(zr[:, :], zr[:, :])
                ev = eL.rearrange("p (j o) -> p j o", o=NO)
                zv = zr.rearrange("p (j a) -> p j a", a=1)
                import concourse.bass as bassm
                ev2, zv2 = bassm.broadcast_tensor_aps(ev, zv)
                nc.vector.tensor_tensor(
                    c16.rearrange("p (j o) -> p j o", o=NO), ev2, zv2,
                    ALU.mult)

            def s_pass():
                """sacc[p,(k,o)] = sum_j c*u_hat ; fold c~ via ones matmul.
                Returns s PSUM [32,160]."""
                for jb in range(0, NJ, JB):
                    t = tmpp.tile([128, JB * OK], dt.bfloat16, tag="t")
                    tv = t.rearrange("p (j k o) -> p j k o", j=JB, k=K)
                    uv = u_hat[:, jb * OK:(jb + JB) * OK].rearrange(
                        "p (j k o) -> p j k o", j=JB, k=K)
                    cv = c16[:, jb * NO:(jb + JB) * NO].rearrange(
                        "p (j a o) -> p j a o", j=JB, a=1)
                    import concourse.bass as bassm
                    uv2, cv2 = bassm.broadcast_tensor_aps(uv, cv)
                    nc.vector.tensor_tensor(tv, uv2, cv2, ALU.mult)
                    jj = JB
                    while jj > 1:
                        h = jj // 2
                        a0 = t.rearrange("p (j f) -> p j f", j=JB)
                        nc.vector.tensor_add(
                            a0[:, 0:h, :], a0[:, 0:h, :], a0[:, h:jj, :])
                        jj = h
                    blk = t[:, 0:OK]
                    if jb == 0:
                        nc.vector.tensor_copy(sacc[:, :], blk)
                    else:
                        nc.vector.tensor_add(sacc[:, :], sacc[:, :], blk)
                nc.vector.tensor_copy(sacc16[:, :], sacc[:, :])
                s_ps = pss.tile([BL, OK], dt.float32, tag="sps")
                nc.tensor.matmul(
                    s_ps[:, :], ones_sb[:, :], sacc16[:, :],
                    start=True, stop=True)
                return s_ps

            # =========== routing ===========
            done = [False]

            def stop_here():
                nc.sync.dma_start(out_d[:, :], biasr_sb[:, :])
                done[0] = True

            if upto <= 0:
                stop_here()
            else:
                tc.strict_bb_all_engine_barrier()
            # pre-observe const DMA queues on DVE/ACT so later ops need <=1 wait
            nc.vector.tensor_copy(touch[:, 0:1], biasr_sb[0:1, 0:1])
            nc.scalar.copy(touch[:, 1:2], biasr_sb[0:1, 1:2])
            # iter 1: c uniform=0.1 -> v1 from s1 (scale 0.1 applied in squash)
            squash_to_vrep(s1_ps, store_out=None)   # store_out=None => scale .1
            if debug:
                nc.sync.dma_start(dbg_u[:, :], u_hat[:, :])
                nc.sync.dma_start(dbg_v1[:, :], vrep[:, :])
            tc.strict_bb_all_engine_barrier()
            if not done[0] and upto <= 1:
                stop_here()
            if not done[0]:
                agreement_pass(first=True)          # L = a1
            if debug:
                nc.sync.dma_start(dbg_L[:, :], L[:, :])
            if not done[0] and upto <= 2:
                stop_here()
            if not done[0]:
                softmax()
                s2 = s_pass()
                squash_to_vrep(s2)                  # v2 -> vrep
            if not done[0] and upto <= 3:
                stop_here()
            if not done[0]:
                agreement_pass(first=False)         # L += a2
            if not done[0]:
                softmax()
                tc.strict_bb_all_engine_barrier()
                s3 = s_pass()
                squash_to_vrep(s3, store_out=True)  # final v -> DRAM

    nc.finalize()
    return nc


def kernel(x, W, bias):
    x = np.asarray(x, dtype=np.float32)
    W = np.asarray(W, dtype=np.float32)
    bias = np.asarray(bias, dtype=np.float32)

    from concourse.bass_utils import run_bass_kernel_spmd

    if "nc" not in _CACHE:
        _CACHE["nc"] = _build_bass()
    nc = _CACHE["nc"]

    in_maps = _pack_inputs(x, W, bias)
    res = run_bass_kernel_spmd(nc, in_maps, core_ids=list(range(NC)))
    _CACHE["last_results"] = res

    out = np.zeros((B, NO, K), dtype=np.float32)
    for cid in range(NC):
        v = res.results[cid]["out_v"]          # [32, 160] in (k,o) order
        out[cid * BL:(cid + 1) * BL] = (
            v.reshape(BL, K, NO).transpose(0, 2, 1))
    return out


if __name__ == "__main__":
    import reference
    inputs = reference.setup_inputs()
    inputs = {k: np.asarray(v) for k, v in inputs.items()}
    expected = np.asarray(reference.reference(**inputs))
    actual = kernel(**inputs)
    err = np.abs(actual - expected).max() / (np.abs(expected).max() + 1e-12)
    print("Relative error:", err)
